# revision 1
# baseline (speedup 1.0000x reference)
"""Causal MHA (B=2, N=2048, D=1024, H=16) on 8 NeuronCores via Bass/Tile.

Sharding: core c = (b, g): b = c // 4 (batch), g = c % 4 (head group of 4
heads = 256 features). Each core computes its Q/K/V projections, causal
attention for its 4 heads, and a partial output projection (its 256 rows of
Wo). The host sums the 4 partials per batch ("unshard" of row-parallel TP).

Layout: activations are feature-major (features on SBUF partitions, sequence
on the free axis), so S^T = K Q^T tiles come out of the PE with k on
partitions and q free and exp() needs no reduction at all. The softmax
denominator falls out of the P@V matmul via a ones column appended to V; the
per-(head, q) normalization uses a reciprocal row broadcast across partitions
through a DRAM bounce. Projections stream x in two d-halves (SBUF partial
sums) so x_q loads overlap the x_kv passes; the output projection is
interleaved into the attention ss loop so PE/DMA stay busy end-to-end.
All matmuls run in float32r (~1.5e-4 rel err, full PE rate at free >= 256).
"""

import numpy as np

import concourse.bass as bass
import concourse.bacc as bacc
import concourse.mybir as mybir
from concourse.tile import TileContext
from concourse.bass_utils import run_bass_kernel_spmd

F32 = mybir.dt.float32
F32R = mybir.dt.float32r
AF = mybir.ActivationFunctionType

B, N, D, H, DH = 2, 2048, 1024, 16, 64
NCORES = 8
GROUPS = 4
HPC = H // GROUPS     # 4 heads per core
FS = HPC * DH         # 256
P = 128
NDT = N // 128        # 16
NSS = N // 512        # 4
DT = D // 128         # 8
FT = FS // 128        # 2
DH2 = DT // 2         # d-tiles per half

_CACHE = {}


def _build(repeat=1, phases="all"):
    nc = bacc.Bacc("TRN2", target_bir_lowering=False, debug=False)

    xqT = nc.dram_tensor("xqT", [D, N], F32R, kind="ExternalInput")
    xkvT = nc.dram_tensor("xkvT", [D, N], F32R, kind="ExternalInput")
    wq = nc.dram_tensor("wq", [D, FS], F32R, kind="ExternalInput")
    wk = nc.dram_tensor("wk", [D, FS], F32R, kind="ExternalInput")
    wv = nc.dram_tensor("wv", [D, FS], F32R, kind="ExternalInput")
    wo = nc.dram_tensor("wo", [FS, D], F32R, kind="ExternalInput")
    bq = nc.dram_tensor("bq", [FS], F32, kind="ExternalInput")
    bk = nc.dram_tensor("bk", [FS], F32, kind="ExternalInput")
    bv = nc.dram_tensor("bv", [1, FS], F32R, kind="ExternalInput")
    bo = nc.dram_tensor("bo", [1, D], F32R, kind="ExternalInput")
    masks = nc.dram_tensor("masks", [P, P], F32R, kind="ExternalInput")
    out = nc.dram_tensor("out_p", [N, D], F32, kind="ExternalOutput")

    with TileContext(nc) as tc:
        with (
            tc.tile_pool(name="const", bufs=1) as cp,
            tc.tile_pool(name="xt", bufs=1) as xp,
            tc.tile_pool(name="acts", bufs=1) as ap_,
            tc.tile_pool(name="ps", bufs=4, space="PSUM") as psp,
            tc.tile_pool(name="pt", bufs=3) as ptp,
            tc.tile_pool(name="small", bufs=4) as smp,
            tc.tile_pool(name="osb", bufs=3) as osp,
            tc.tile_pool(name="dsc", bufs=4, space="DRAM") as dsp,
        ):
            wo_sb = cp.tile([P, FT, D], F32R, tag="wo")
            bqk_sb = cp.tile([P, 2, 2], F32, tag="bqk")
            bv_sb = cp.tile([1, FS], F32R, tag="bv")
            bo_sb = cp.tile([1, D], F32R, tag="bo")
            tri_sb = cp.tile([P, P], F32R, tag="mask")
            ones_r = cp.tile([1, P], F32R, tag="ones")
            ones_f = cp.tile([P, HPC], F32, tag="ones_f")
            ones_fr = cp.tile([1, P], F32, tag="ones_fr")
            bo_rep = cp.tile([P, D], F32, tag="bo_rep")
            bv_rep = cp.tile([P, FS], F32, tag="bv_rep")

            nc.sync.dma_start(out=wo_sb, in_=wo.ap().rearrange("(t p) f -> p t f", p=P))
            nc.sync.dma_start(out=bqk_sb[:, 0, :], in_=bk.ap().rearrange("(t p) -> p t", p=P))
            nc.sync.dma_start(out=bqk_sb[:, 1, :], in_=bq.ap().rearrange("(t p) -> p t", p=P))
            nc.sync.dma_start(out=bv_sb, in_=bv.ap())
            nc.sync.dma_start(out=bo_sb, in_=bo.ap())
            nc.sync.dma_start(out=tri_sb, in_=masks.ap())
            nc.vector.memset(ones_f, 1.0)
            nc.vector.memset(ones_fr, 1.0)
            nc.vector.tensor_copy(ones_r, ones_fr)

            # one-time replicated bias tiles (replaces per-tile K=1 matmuls,
            # which measure ~1.1us each on HW)
            ps_rep = psp.tile([P, 512], F32, tag="ps", name="ps_brep")
            nc.tensor.matmul(ps_rep, ones_r[:, 0:P], bo_sb[:, 0:512], start=True, stop=True)
            nc.vector.tensor_copy(bo_rep[:, 0:512], ps_rep)
            ps_rep2 = psp.tile([P, 512], F32, tag="ps", name="ps_brep2")
            nc.tensor.matmul(ps_rep2, ones_r[:, 0:P], bo_sb[:, 512:1024], start=True, stop=True)
            nc.vector.tensor_copy(bo_rep[:, 512:1024], ps_rep2)
            ps_rep3 = psp.tile([P, 512], F32, tag="ps", name="ps_brep3")
            nc.tensor.matmul(ps_rep3[:, 0:FS], ones_r[:, 0:P], bv_sb, start=True, stop=True)
            nc.vector.tensor_copy(bv_rep, ps_rep3[:, 0:FS])

            kt_all = [ap_.tile([P, N], F32R, tag=f"kt{f}", name=f"kt{f}") for f in range(FT)]
            qt_all = [ap_.tile([P, N], F32R, tag=f"qt{f}", name=f"qt{f}") for f in range(FT)]
            v_sb = [ap_.tile([P, HPC, DH + 1], F32R, tag=f"v{st}", name=f"v{st}") for st in range(NDT)]
            ot_all = [ap_.tile([P, N], F32R, tag=f"ot{f}", name=f"ot{f}") for f in range(FT)]

            def emit_body():
                # ---- projections, streamed in two d-halves ----
                for half in range(2):
                    d0 = half * DH2
                    wk_sb = cp.tile([P, DH2, FS], F32R, tag="w", bufs=3, name="wk_h")
                    nc.sync.dma_start(out=wk_sb, in_=wk.ap().rearrange("(t p) f -> p t f", p=P)[:, d0:d0 + DH2, :])
                    wv_sb = cp.tile([P, DH2, FS], F32R, tag="w", bufs=3, name="wv_h")
                    nc.sync.dma_start(out=wv_sb, in_=wv.ap().rearrange("(t p) f -> p t f", p=P)[:, d0:d0 + DH2, :])
                    wq_sb = cp.tile([P, DH2, FS], F32R, tag="w", bufs=3, name="wq_h")
                    nc.sync.dma_start(out=wq_sb, in_=wq.ap().rearrange("(t p) f -> p t f", p=P)[:, d0:d0 + DH2, :])
                    xkv_t, xq_t = [], []
                    for i in range(DH2):
                        d = d0 + i
                        t = xp.tile([P, N], F32R, tag=f"xkv{i}", name=f"xkv{i}")
                        nc.sync.dma_start(out=t, in_=xkvT.ap()[d * P:(d + 1) * P, :])
                        xkv_t.append(t)
                    for i in range(DH2):
                        d = d0 + i
                        t = xp.tile([P, N], F32R, tag=f"xq{i}", name=f"xq{i}")
                        nc.sync.dma_start(out=t, in_=xqT.ap()[d * P:(d + 1) * P, :])
                        xq_t.append(t)

                    # K pass
                    for ft in range(FT):
                        for ss in range(NSS):
                            ps = psp.tile([P, 512], F32, tag="ps", name="ps_k")
                            for i in range(DH2):
                                nc.tensor.matmul(
                                    ps,
                                    wk_sb[:, i, ft * P:(ft + 1) * P],
                                    xkv_t[i][:, ss * 512:(ss + 1) * 512],
                                    start=(i == 0),
                                    stop=(i == DH2 - 1),
                                )
                            dst = kt_all[ft][:, ss * 512:(ss + 1) * 512]
                            if half == 0:
                                nc.scalar.activation(dst, ps, AF.Identity, bias=bqk_sb[:, 0, ft:ft + 1])
                            else:
                                nc.vector.tensor_add(dst, dst, ps)
                    # V pass
                    for st in range(NDT):
                        psv = psp.tile([P, 512], F32, tag="ps", name="ps_v")
                        for i in range(DH2):
                            nc.tensor.matmul(
                                psv[:, 0:FS],
                                xkv_t[i][:, st * P:(st + 1) * P],
                                wv_sb[:, i, :],
                                start=(i == 0),
                                stop=(i == DH2 - 1),
                            )
                        vdst = v_sb[st][:, :, 0:DH]
                        psv_v = psv[:, 0:FS].rearrange("p (h c) -> p h c", h=HPC)
                        if half == 0:
                            nc.vector.tensor_add(vdst, psv_v, bv_rep.rearrange("p (h c) -> p h c", h=HPC))
                            nc.vector.tensor_copy(v_sb[st][:, :, DH], ones_f)
                        else:
                            nc.vector.tensor_add(vdst, vdst, psv_v)
                    # Q pass
                    for ft in range(FT):
                        for ss in range(NSS):
                            ps = psp.tile([P, 512], F32, tag="ps", name="ps_q")
                            for i in range(DH2):
                                nc.tensor.matmul(
                                    ps,
                                    wq_sb[:, i, ft * P:(ft + 1) * P],
                                    xq_t[i][:, ss * 512:(ss + 1) * 512],
                                    start=(i == 0),
                                    stop=(i == DH2 - 1),
                                )
                            dst = qt_all[ft][:, ss * 512:(ss + 1) * 512]
                            if half == 0:
                                nc.scalar.activation(dst, ps, AF.Identity, bias=bqk_sb[:, 1, ft:ft + 1])
                            else:
                                nc.vector.tensor_add(dst, dst, ps)

                if phases == "proj":
                    row = 0
                    for tset in (kt_all, qt_all):
                        for tt in tset:
                            for half in range(2):
                                nc.sync.dma_start(
                                    out=out.ap()[row * P:(row + 1) * P, :],
                                    in_=tt[:, half * D:(half + 1) * D].bitcast(F32),
                                )
                                row += 1
                    for st in range(NDT):
                        rr = 8 + st % 8
                        nc.sync.dma_start(
                            out=out.ap()[rr * P:(rr + 1) * P, 0:HPC * (DH + 1)],
                            in_=v_sb[st].rearrange("p h c -> p (h c)").bitcast(F32),
                        )
                    return

                def emit_oproj(ss_):
                    for qt in range(4 * ss_, 4 * ss_ + 4):
                        o_sb = osp.tile([P, D], F32, tag="osb", name="o_sb")
                        for os_ in range(2):
                            ps_o = psp.tile([P, 512], F32, tag="ps", name="ps_o")
                            for ft in range(FT):
                                nc.tensor.matmul(
                                    ps_o,
                                    ot_all[ft][:, qt * P:(qt + 1) * P],
                                    wo_sb[:, ft, os_ * 512:(os_ + 1) * 512],
                                    start=(ft == 0),
                                    stop=(ft == FT - 1),
                                )
                            nc.vector.tensor_add(
                                o_sb[:, os_ * 512:(os_ + 1) * 512],
                                ps_o,
                                bo_rep[:, os_ * 512:(os_ + 1) * 512],
                            )
                        nc.sync.dma_start(out=out.ap()[qt * P:(qt + 1) * P, :], in_=o_sb)

                # ---- attention (2 heads packed per ST step) + interleaved O-proj ----
                for ss in range(NSS):
                    n_kt = 4 * ss + 4
                    for ft in range(FT):
                        otp = [
                            psp.tile([P, 512], F32, tag="ps", name=f"ps_ot{hh}")
                            for hh in range(2)
                        ]
                        for kt in range(n_kt):
                            st2 = psp.tile([P, 1024], F32, tag="ps2", bufs=2, name="ps_st2")
                            ptt = ptp.tile([P, 1024], F32R, tag="pt", name="ptt")
                            for hh in range(2):
                                nc.tensor.matmul(
                                    st2[:, hh * 512:(hh + 1) * 512],
                                    kt_all[ft][hh * 64:(hh + 1) * 64, kt * P:(kt + 1) * P],
                                    qt_all[ft][hh * 64:(hh + 1) * 64, ss * 512:(ss + 1) * 512],
                                    start=True, stop=True,
                                )
                            nc.scalar.activation(ptt, st2, AF.Exp, scale=0.125)
                            dk = (kt - 4 * ss) * P
                            if dk >= 0:
                                for hh in range(2):
                                    base = hh * 512
                                    if dk > 0:
                                        nc.vector.tensor_scalar_mul(
                                            ptt[:, base:base + dk],
                                            ptt[:, base:base + dk],
                                            0.0,
                                        )
                                    nc.vector.tensor_mul(
                                        ptt[:, base + dk:base + dk + P],
                                        ptt[:, base + dk:base + dk + P],
                                        tri_sb,
                                    )
                            for hh in range(2):
                                nc.tensor.matmul(
                                    otp[hh][0:DH + 1, :],
                                    v_sb[kt][:, ft * 2 + hh, :],
                                    ptt[:, hh * 512:(hh + 1) * 512],
                                    start=(kt == 0),
                                    stop=(kt == n_kt - 1),
                                )
                        # normalization: reciprocal row, broadcast via DRAM bounce
                        rept = smp.tile([DH + 1, 1024], F32R, tag="rep_sb", bufs=2, name="rept")
                        recip = rept[DH:DH + 1, :]
                        rep_sb = rept[0:DH, :]
                        with nc.allow_low_precision(reason="softmax reciprocal"):
                            nc.vector.reciprocal(recip[:, 0:512], otp[0][DH:DH + 1, :])
                            nc.vector.reciprocal(recip[:, 512:1024], otp[1][DH:DH + 1, :])
                        dscr = dsp.tile([1, 1024], F32R, tag="dscr", name="dscr")
                        nc.sync.dma_start(out=dscr, in_=recip)
                        rep_bcast = bass.AP(
                            tensor=dscr.tensor,
                            offset=dscr.offset,
                            ap=[[0, DH]] + [list(x) for x in dscr.ap[1:]],
                        )
                        nc.sync.dma_start(out=rep_sb, in_=rep_bcast)
                        for hh in range(2):
                            row = hh * 64
                            nc.vector.tensor_mul(
                                ot_all[ft][row:row + 64, ss * 512:(ss + 1) * 512],
                                otp[hh][0:DH, :],
                                rep_sb[:, hh * 512:(hh + 1) * 512],
                            )

                    if phases == "proj+attn":
                        continue
                    # O-proj deferred one ss so the PE never waits on the
                    # normalize chain of the slice it is about to project
                    if ss > 0:
                        emit_oproj(ss - 1)
                    if ss == NSS - 1:
                        emit_oproj(ss)

                if phases == "proj+attn":
                    row = 0
                    for tt in ot_all:
                        for half in range(2):
                            nc.sync.dma_start(
                                out=out.ap()[row * P:(row + 1) * P, :],
                                in_=tt[:, half * D:(half + 1) * D].bitcast(F32),
                            )
                            row += 1
                    return

            if repeat == 1:
                emit_body()
            else:
                with tc.For_i(0, repeat, 1):
                    emit_body()

    nc.compile()
    return nc


def _shard_inputs(x_q, x_kv, Wq, bq_, Wk, bk_, Wv, bv_, Wo, bo_):
    pp_, ff = np.meshgrid(np.arange(P), np.arange(P), indexing="ij")
    mask = (ff >= pp_).astype(np.float32)
    in_maps = []
    for c in range(NCORES):
        b, g = c // GROUPS, c % GROUPS
        sl = slice(g * FS, (g + 1) * FS)
        in_maps.append({
            "xqT": np.ascontiguousarray(x_q[b].T),
            "xkvT": np.ascontiguousarray(x_kv[b].T),
            "wq": np.ascontiguousarray(Wq[:, sl]),
            "wk": np.ascontiguousarray(Wk[:, sl]),
            "wv": np.ascontiguousarray(Wv[:, sl]),
            "wo": np.ascontiguousarray(Wo[sl, :]),
            "bq": np.ascontiguousarray(bq_[sl]),
            "bk": np.ascontiguousarray(bk_[sl]),
            "bv": np.ascontiguousarray(bv_[sl]).reshape(1, FS),
            "bo": (bo_ if g == 0 else np.zeros_like(bo_)).reshape(1, D),
            "masks": mask,
        })
    return in_maps


def kernel(x_q, x_kv, Wq, bq, Wk, bk, Wv, bv, Wo, bo):
    x_q = np.asarray(x_q, dtype=np.float32)
    x_kv = np.asarray(x_kv, dtype=np.float32)
    if "nc" not in _CACHE:
        _CACHE["nc"] = _build()
    nc = _CACHE["nc"]
    in_maps = _shard_inputs(
        x_q, x_kv,
        np.asarray(Wq, np.float32), np.asarray(bq, np.float32),
        np.asarray(Wk, np.float32), np.asarray(bk, np.float32),
        np.asarray(Wv, np.float32), np.asarray(bv, np.float32),
        np.asarray(Wo, np.float32), np.asarray(bo, np.float32),
    )
    res = run_bass_kernel_spmd(nc, in_maps, core_ids=list(range(NCORES)))
    out = np.zeros((B, N, D), dtype=np.float32)
    for c in range(NCORES):
        out[c // GROUPS] += res.results[c]["out_p"]
    return out



# revision 6
# speedup vs baseline: 1.2006x; 1.2006x over previous
"""Causal MHA (B=2, N=2048, D=1024, H=16) on 8 NeuronCores via Bass/Tile.

Sharding: core c = (b, g): b = c // 4 (batch), g = c % 4 (head group of 4
heads = 256 features). Row-parallel O-proj; host sums the 4 bf16 partials
per batch and adds bo.

Design (vs the fp32r baseline, ~305us -> ~217us locally):
- all matmul operands bf16 (measured PE rate 0.661 ns/row regardless of
  bf16/fp32r/fp8-DoubleRow; bf16 halves DMA+SBUF and lifts the fp32r
  free>=256 restriction, enabling causal trimming)
- projections stream x in two 1024-col seq chunks; K/V/Q accumulate the
  full d=1024 contraction in PSUM once (no second-pass adds); K/Q psum
  groups borrow the attention score banks ([P,1024] x2) with one fused
  bias copy-out per ft
- emission order projc0 -> attn(ss0,ss1) -> projc1 -> attn(ss2,ss3):
  the exp stream of early attention hides under chunk-1 projections,
  whose K/Q copy-outs go to DVE so ACT stays exp-only
- causal trimming at 128-col granularity: score matmuls, exp, and PV all
  skip fully-masked q-columns of diagonal blocks (PE rows 81920->69632
  per pass; exp elements -30%); intra-block triangle masked on the Pool
  engine (SBUF bf16), off the DVE/ACT critical paths
- exp of both packed heads in ONE activation instr via a strided AP view
- softmax denominator via a ones-column appended to V; normalization
  copies numerator+denominator to SBUF immediately (frees the PSUM
  accumulators), then reciprocal + DRAM-bounce broadcast + bf16 muls run
  off the critical path, deferred one q-chunk
- O-proj emitted as deferred filler between attention kt-steps, two
  dout-tiles per [P,1024] psum tile shared with the score-bank rotation
"""

import numpy as np
import ml_dtypes

import concourse.bass as bass
import concourse.bacc as bacc
import concourse.mybir as mybir
from concourse.tile import TileContext
from concourse.bass_utils import run_bass_kernel_spmd

F32 = mybir.dt.float32
BF16 = mybir.dt.bfloat16
AF = mybir.ActivationFunctionType
NPBF16 = ml_dtypes.bfloat16

B, N, D, H, DH = 2, 2048, 1024, 16, 64
NCORES = 8
GROUPS = 4
HPC = H // GROUPS     # 4 heads per core
FS = HPC * DH         # 256
P = 128
NDT = N // 128        # 16 seq tiles of 128
NSS = N // 512        # 4 q-chunks of 512
DT = D // 128         # 8 d-tiles
FT = FS // 128        # 2 feature tiles of 128 (2 heads each)
NCH = 2               # seq chunks of 1024 for projection streaming
CHW = N // NCH        # 1024

_CACHE = {}


def _build(repeat=1, phases="all"):
    nc = bacc.Bacc("TRN2", target_bir_lowering=False, debug=False)

    xqT = nc.dram_tensor("xqT", [P, DT, N], BF16, kind="ExternalInput")
    xkvT = nc.dram_tensor("xkvT", [P, DT, N], BF16, kind="ExternalInput")
    wq = nc.dram_tensor("wq", [P, DT, FS], BF16, kind="ExternalInput")
    wk = nc.dram_tensor("wk", [P, DT, FS], BF16, kind="ExternalInput")
    wv = nc.dram_tensor("wv", [P, DT, FS], BF16, kind="ExternalInput")
    wo = nc.dram_tensor("wo", [P, FT, D], BF16, kind="ExternalInput")
    bqk = nc.dram_tensor("bqk", [P, 2, FT], F32, kind="ExternalInput")
    bv_rep = nc.dram_tensor("bv_rep", [P, FS], BF16, kind="ExternalInput")
    masks = nc.dram_tensor("masks", [P, P], BF16, kind="ExternalInput")
    out = nc.dram_tensor("out_p", [P, DT, N], BF16, kind="ExternalOutput")

    with TileContext(nc) as tc:
        with (
            tc.tile_pool(name="const", bufs=1) as cp,
            tc.tile_pool(name="xt", bufs=1) as xp,
            tc.tile_pool(name="acts", bufs=1) as ap_,
            tc.tile_pool(name="accps", bufs=1, space="PSUM") as accp,
            tc.tile_pool(name="stps", bufs=3, space="PSUM") as stp,
            tc.tile_pool(name="pt", bufs=4) as ptp,
            tc.tile_pool(name="small", bufs=2) as smp,
            tc.tile_pool(name="osb", bufs=3) as osp,
            tc.tile_pool(name="dsc", bufs=4, space="DRAM") as dsp,
        ):
            wo_sb = cp.tile([P, FT, D], BF16, tag="wo")
            bqk_sb = cp.tile([P, 2, FT], F32, tag="bqk")
            bvr_sb = cp.tile([P, FS], BF16, tag="bvr")
            tri_sb = cp.tile([P, P], BF16, tag="mask")

            nc.sync.dma_start(out=wo_sb, in_=wo.ap())
            nc.sync.dma_start(out=bqk_sb, in_=bqk.ap())
            nc.sync.dma_start(out=bvr_sb, in_=bv_rep.ap())
            nc.sync.dma_start(out=tri_sb, in_=masks.ap())

            kt_all = [ap_.tile([P, N], BF16, tag=f"kt{f}", name=f"kt{f}") for f in range(FT)]
            qt_all = [ap_.tile([P, N], BF16, tag=f"qt{f}", name=f"qt{f}") for f in range(FT)]
            ot_all = [ap_.tile([P, N], BF16, tag=f"ot{f}", name=f"ot{f}") for f in range(FT)]
            v_sb = [ap_.tile([P, HPC, DH + 1], BF16, tag=f"v{st}", name=f"v{st}") for st in range(NDT)]
            # constant ones column for the softmax denominator (col DH);
            # projection copy-outs only touch cols 0..DH so this persists
            for st in range(NDT):
                nc.vector.memset(v_sb[st][:, :, DH], 1.0)

            def acc_tile(name):
                return accp.tile([P, 512], F32, tag="otp", bufs=2, name=name)

            def emit_proj_loads():
                wk_sb = cp.tile([P, DT, FS], BF16, tag="wk", bufs=2, name="wk_sb")
                nc.sync.dma_start(out=wk_sb, in_=wk.ap())
                wv_sb = cp.tile([P, DT, FS], BF16, tag="wv", bufs=2, name="wv_sb")
                nc.sync.dma_start(out=wv_sb, in_=wv.ap())
                xkv_t, xq_t = [], []
                # chunk 0 is on the critical path to the first matmul:
                # split its loads at 512-col granularity so compute starts
                # as soon as the first half lands
                t = xp.tile([P, DT, CHW], BF16, tag="xkv0", name="xkv0")
                for h2 in range(2):
                    nc.sync.dma_start(
                        out=t[:, :, h2 * 512:(h2 + 1) * 512],
                        in_=xkvT.ap()[:, :, h2 * 512:(h2 + 1) * 512],
                    )
                xkv_t.append(t)
                wq_sb = cp.tile([P, DT, FS], BF16, tag="wq", bufs=2, name="wq_sb")
                nc.sync.dma_start(out=wq_sb, in_=wq.ap())
                t = xp.tile([P, DT, CHW], BF16, tag="xq0", name="xq0")
                for h2 in range(2):
                    nc.sync.dma_start(
                        out=t[:, :, h2 * 512:(h2 + 1) * 512],
                        in_=xqT.ap()[:, :, h2 * 512:(h2 + 1) * 512],
                    )
                xq_t.append(t)
                for c in (1,):
                    t = xp.tile([P, DT, CHW], BF16, tag=f"xkv{c}", name=f"xkv{c}")
                    nc.sync.dma_start(out=t, in_=xkvT.ap()[:, :, c * CHW:(c + 1) * CHW])
                    xkv_t.append(t)
                    t = xp.tile([P, DT, CHW], BF16, tag=f"xq{c}", name=f"xq{c}")
                    nc.sync.dma_start(out=t, in_=xqT.ap()[:, :, c * CHW:(c + 1) * CHW])
                    xq_t.append(t)
                return wk_sb, wv_sb, wq_sb, xkv_t, xq_t

            def emit_proj_chunk(c, wk_sb, wv_sb, wq_sb, xkv_t, xq_t, on_act):
                # K/Q PSUM groups use the (idle during proj) st2 banks:
                # one [P, 1024] group per ft, single fused copy-out
                def kq_pass(w_sb, x_t, dst_all, kq):
                    for ft in range(FT):
                        ps = stp.tile([P, 1024], F32, tag="st2", name="ps_kq")
                        for sub in range(2):
                            for i in range(DT):
                                nc.tensor.matmul(
                                    ps[:, sub * 512:(sub + 1) * 512],
                                    w_sb[:, i, ft * P:(ft + 1) * P],
                                    x_t[c][:, i, sub * 512:(sub + 1) * 512],
                                    start=(i == 0),
                                    stop=(i == DT - 1),
                                )
                        dst = dst_all[ft][:, c * CHW:(c + 1) * CHW]
                        if on_act:
                            nc.scalar.activation(
                                dst, ps, AF.Identity, bias=bqk_sb[:, kq, ft:ft + 1]
                            )
                        else:
                            # DVE path: keeps ACT free for attention exp
                            # running concurrently with this projection chunk
                            nc.vector.tensor_scalar_add(
                                dst, ps, bqk_sb[:, kq, ft:ft + 1]
                            )

                kq_pass(wk_sb, xkv_t, kt_all, 0)
                # V: out seq-major [128s, 256f]
                for st in range(CHW // P):
                    ps = acc_tile("ps_v")
                    for i in range(DT):
                        nc.tensor.matmul(
                            ps[:, 0:FS],
                            xkv_t[c][:, i, st * P:(st + 1) * P],
                            wv_sb[:, i, :],
                            start=(i == 0),
                            stop=(i == DT - 1),
                        )
                    st_g = c * (CHW // P) + st
                    nc.vector.tensor_add(
                        v_sb[st_g][:, :, 0:DH],
                        ps[:, 0:FS].rearrange("p (h d) -> p h d", h=HPC),
                        bvr_sb.rearrange("p (h d) -> p h d", h=HPC),
                    )
                kq_pass(wq_sb, xq_t, qt_all, 1)

            def dump_proj():
                row = 0
                for tset in (kt_all, qt_all):
                    for tt in tset:
                        nc.sync.dma_start(out=out.ap()[:, row, :], in_=tt)
                        row += 1
                for st in range(NDT):
                    rr = 4 + st // 7
                    c0 = (st % 7) * (HPC * (DH + 1))
                    nc.sync.dma_start(
                        out=out.ap()[:, rr, c0: c0 + HPC * (DH + 1)],
                        in_=v_sb[st].rearrange("p h c -> p (h c)"),
                    )

            def oproj_items(ss_):
                # one item = two dout-tiles of O-proj for q-chunk ss_,
                # packed in one [P, 1024] tile from the st2 rotation
                items = []
                for dp in range(DT // 2):
                    def emit(dp=dp):
                        ps = stp.tile([P, 1024], F32, tag="st2", name="ps_o")
                        for half in range(2):
                            dt_ = dp * 2 + half
                            for ft in range(FT):
                                nc.tensor.matmul(
                                    ps[:, half * 512:(half + 1) * 512],
                                    wo_sb[:, ft, dt_ * P:(dt_ + 1) * P],
                                    ot_all[ft][:, ss_ * 512:(ss_ + 1) * 512],
                                    start=(ft == 0),
                                    stop=(ft == FT - 1),
                                )
                        o_sb = osp.tile([P, 1024], BF16, tag="osb", name="o_sb")
                        nc.vector.tensor_copy(o_sb, ps)
                        nc.sync.dma_start(
                            out=out.ap()[:, dp * 2, ss_ * 512:(ss_ + 1) * 512],
                            in_=o_sb[:, 0:512],
                        )
                        nc.sync.dma_start(
                            out=out.ap()[:, dp * 2 + 1, ss_ * 512:(ss_ + 1) * 512],
                            in_=o_sb[:, 512:1024],
                        )
                    items.append(emit)
                return items

            def emit_attn_ss(ss, pending):
                if True:
                    n_kt = 4 * ss + 4
                    # spread pending O-proj items across this ss's kt steps
                    n_steps = 2 * n_kt
                    fill_every = max(1, n_steps // len(pending)) if pending else 0
                    step = 0
                    for ft in range(FT):
                        otp = [
                            accp.tile([P, 512], F32, tag="otp", bufs=2, name=f"ps_ot{hh}")
                            for hh in range(2)
                        ]
                        for kt in range(n_kt):
                            diag = kt - 4 * ss
                            q0 = diag * P if diag > 0 else 0
                            st2 = stp.tile([P, 1024], F32, tag="st2", name="ps_st2")
                            st2v = st2.rearrange("p (h q) -> p h q", h=2)
                            for hh in range(2):
                                nc.tensor.matmul(
                                    st2v[:, hh, q0:512],
                                    kt_all[ft][hh * 64:(hh + 1) * 64, kt * P:(kt + 1) * P],
                                    qt_all[ft][hh * 64:(hh + 1) * 64, ss * 512 + q0:(ss + 1) * 512],
                                    start=True, stop=True,
                                )
                            ptt = ptp.tile([P, 1024], BF16, tag="pt", name="ptt")
                            pttv = ptt.rearrange("p (h q) -> p h q", h=2)
                            nc.scalar.activation(
                                pttv[:, :, q0:512], st2v[:, :, q0:512], AF.Exp, scale=0.125
                            )
                            if diag >= 0:
                                # intra-block triangle on the Pool engine
                                # (SBUF-only op; keeps DVE free)
                                for hh in range(2):
                                    nc.gpsimd.tensor_mul(
                                        pttv[:, hh, q0:q0 + P],
                                        pttv[:, hh, q0:q0 + P],
                                        tri_sb,
                                    )
                            for hh in range(2):
                                # trimmed moving span [q0:512]: fully-masked
                                # q-columns of diagonal blocks never touch PE
                                nc.tensor.matmul(
                                    otp[hh][0:DH + 1, q0:512],
                                    v_sb[kt][:, ft * 2 + hh, :],
                                    pttv[:, hh, q0:512],
                                    start=(kt == 0),
                                    stop=(kt == n_kt - 1),
                                    skip_group_check=True,
                                )
                            step += 1
                            if pending and fill_every and step % fill_every == 0:
                                pending.pop(0)()
                        # copy numerator+denominator to SBUF immediately so the
                        # otp PSUM accumulators free up for the next (ss, ft);
                        # the rest of the normalize chain runs off-critical-path
                        num_sb = smp.tile([DH + 1, 1024], BF16, tag="num", name="num_sb")
                        nc.vector.tensor_copy(
                            num_sb[0:DH + 1, 0:512], otp[0][0:DH + 1, :]
                        )
                        nc.vector.tensor_copy(
                            num_sb[0:DH + 1, 512:1024], otp[1][0:DH + 1, :]
                        )
                        rept = smp.tile([DH + 1, 1024], BF16, tag="rep", name="rept")
                        recip = rept[DH:DH + 1, :]
                        rep_sb = rept[0:DH, :]
                        with nc.allow_low_precision(reason="softmax reciprocal"):
                            nc.vector.reciprocal(recip, num_sb[DH:DH + 1, :])
                        dscr = dsp.tile([1, 1024], BF16, tag="dscr", name="dscr")
                        nc.sync.dma_start(out=dscr, in_=recip)
                        rep_bcast = bass.AP(
                            tensor=dscr.tensor,
                            offset=dscr.offset,
                            ap=[[0, DH]] + [list(x) for x in dscr.ap[1:]],
                        )
                        nc.sync.dma_start(out=rep_sb, in_=rep_bcast)
                        for hh in range(2):
                            nc.vector.tensor_mul(
                                ot_all[ft][hh * 64:(hh + 1) * 64, ss * 512:(ss + 1) * 512],
                                num_sb[0:DH, hh * 512:(hh + 1) * 512],
                                rep_sb[:, hh * 512:(hh + 1) * 512],
                            )
                    while pending:
                        pending.pop(0)()
                    if phases != "proj+attn":
                        pending = oproj_items(ss)
                    return pending

            def dump_attn():
                for r, tt in enumerate(ot_all):
                    nc.sync.dma_start(out=out.ap()[:, r, :], in_=tt)

            def emit_body():
                wk_sb, wv_sb, wq_sb, xkv_t, xq_t = emit_proj_loads()
                emit_proj_chunk(0, wk_sb, wv_sb, wq_sb, xkv_t, xq_t, on_act=True)
                if phases == "proj":
                    emit_proj_chunk(1, wk_sb, wv_sb, wq_sb, xkv_t, xq_t, on_act=True)
                    dump_proj()
                    return
                # attention over the first x chunk (ss0, ss1) runs while the
                # second projection chunk streams; its K/Q copy-outs go to DVE
                # so ACT is free for the concurrent exp work
                pending = emit_attn_ss(0, [])
                pending = emit_attn_ss(1, pending)
                emit_proj_chunk(1, wk_sb, wv_sb, wq_sb, xkv_t, xq_t, on_act=False)
                pending = emit_attn_ss(2, pending)
                pending = emit_attn_ss(3, pending)
                while pending:
                    pending.pop(0)()
                if phases == "proj+attn":
                    dump_attn()

            if repeat == 1:
                emit_body()
            else:
                with tc.For_i(0, repeat, 1):
                    emit_body()

    nc.compile()
    return nc


def _to_tiled(a, tiles):
    """[R, C] -> [128, R//128, C] with row r = t*128+p -> [p, t, :]"""
    r, c = a.shape
    return np.ascontiguousarray(
        a.reshape(tiles, P, c).transpose(1, 0, 2)
    )


def _shard_inputs(x_q, x_kv, Wq, bq_, Wk, bk_, Wv, bv_, Wo, bo_):
    pp_, ff = np.meshgrid(np.arange(P), np.arange(P), indexing="ij")
    mask = (ff >= pp_).astype(NPBF16)
    xq_b = [
        _to_tiled(x_q[b].astype(NPBF16).T, DT) for b in range(B)
    ]
    xkv_b = [
        _to_tiled(x_kv[b].astype(NPBF16).T, DT) for b in range(B)
    ]
    in_maps = []
    for c in range(NCORES):
        b, g = c // GROUPS, c % GROUPS
        sl = slice(g * FS, (g + 1) * FS)
        bqk_h = np.zeros((P, 2, FT), np.float32)
        for ft in range(FT):
            bqk_h[:, 0, ft] = bk_[g * FS + ft * P: g * FS + (ft + 1) * P]
            bqk_h[:, 1, ft] = bq_[g * FS + ft * P: g * FS + (ft + 1) * P]
        in_maps.append({
            "xqT": xq_b[b],
            "xkvT": xkv_b[b],
            "wq": _to_tiled(Wq[:, sl].astype(NPBF16), DT),
            "wk": _to_tiled(Wk[:, sl].astype(NPBF16), DT),
            "wv": _to_tiled(Wv[:, sl].astype(NPBF16), DT),
            "wo": _to_tiled(Wo[sl, :].astype(NPBF16), FT),
            "bqk": bqk_h,
            "bv_rep": np.ascontiguousarray(
                np.broadcast_to(bv_[sl].astype(NPBF16), (P, FS))
            ),
            "masks": mask,
        })
    return in_maps


def kernel(x_q, x_kv, Wq, bq, Wk, bk, Wv, bv, Wo, bo):
    x_q = np.asarray(x_q, dtype=np.float32)
    x_kv = np.asarray(x_kv, dtype=np.float32)
    if "nc" not in _CACHE:
        _CACHE["nc"] = _build()
    nc = _CACHE["nc"]
    in_maps = _shard_inputs(
        x_q, x_kv,
        np.asarray(Wq, np.float32), np.asarray(bq, np.float32),
        np.asarray(Wk, np.float32), np.asarray(bk, np.float32),
        np.asarray(Wv, np.float32), np.asarray(bv, np.float32),
        np.asarray(Wo, np.float32), np.asarray(bo, np.float32),
    )
    res = run_bass_kernel_spmd(nc, in_maps, core_ids=list(range(NCORES)))
    out = np.zeros((B, N, D), dtype=np.float32)
    for c in range(NCORES):
        # out_p [128, 8, N] -> [D, N] -> [N, D]
        op = res.results[c]["out_p"].astype(np.float32)
        out[c // GROUPS] += op.transpose(1, 0, 2).reshape(D, N).T
    out += np.asarray(bo, np.float32)
    return out
